# revision 16
# baseline (speedup 1.0000x reference)
"""Trainium2 Bass kernel for nn_CausalAttention (B=2, S=2048, D=1024, H=16).

Sharding: tensor-parallel over heads (4 groups of 4 heads) x data-parallel
over batch (2), on 8 NeuronCores. Core c handles batch b = c // 4 and head
group g = c % 4 (heads 4g..4g+3, i.e. d_model columns 256g..256g+256).

Each core computes, fully on-device in bf16 (f32 PSUM accumulation):
  Q^T, K^T (d_out on partitions) and V (s on partitions, ones column
  appended) for its head slice, projected incrementally per q-chunk so
  projection matmuls overlap the (exp-latency-bound) attention pipeline;
  transposed score tiles (k on partitions, q free) trimmed to the causal
  region; P^T = exp(S^T / 8) with causal masking on GPSIMD (no max
  subtraction -- scores are O(1) here); unnormalized attention out^T[dh, q]
  via V-stationary matmuls where the ones column yields the softmax
  denominator for free; normalization by the broadcast fast-approx
  reciprocal of the denominator row; then a partial out = attn @ Wo_slice,
  interleaved per q-chunk.

Score matmuls contract over dh=64 only, so the two heads of a W-column
half (partitions 0-63 / 64-127) are emitted as ADJACENT matmul pairs:
their auto-derived PE tile_positions (rows 0-63 vs 64-127) let the two
K=64 matmuls run concurrently in the PE array (row tiling), halving the
score-phase PE time. A zero-filled scratch feeds a ~9us burst of dummy
matmuls at program start so the PE is continuously busy from ~8us on and
the HAM clock-gate releases (2.4 GHz) before the first real chains run --
with no >3.4us PE gap afterwards the clock never re-throttles mid-run.
The first-needed input pieces (wq/wk ci0, the xf quarters) are spread
across all three DMA rings, and chunk-0 projection chains accumulate kc
in DMA-arrival order (sync: 0-3, gpsimd: 6-7, scalar-ring xf45 last) so
the first chains never stall mid-accumulation.

DMA strategy: inputs are pre-arranged host-side so every DMA has 2-4KB
contiguous per-partition lines, split into ~0.25-0.5MB pieces issued in
first-needed order across the three DMA rings (sync, scalar, gpsimd).
The output is written in bf16 (host accumulates partials in f32), one DMA
per 128-row tile, alternating sync/gpsimd rings.

Tail: the final q-chunk's last head splits its P@V into two column-slice
chains, both pipelined into the score-group loop so every PSUM write
lands ~1.5us after the last exp; the three remaining normalizes are
emitted copies/recips-first so the gpsimd broadcasts chain while the PE
streams the out-projection tiles, whose psum->sbuf bf16 casts are split
between the vector and gpsimd engines.

Host-side glue (sharding/gather): x is pre-transposed per batch, weights are
pre-sliced and cast to bf16; the 4 partial outputs per batch are summed and
bo + bv @ Wo (the V-bias contribution, exact since softmax rows sum to 1)
is added.
"""

import sys

for _p in ("/opt/trn_rl_repo",):
    if _p not in sys.path:
        sys.path.append(_p)

import ml_dtypes
import numpy as np

import concourse.bass as bass
import concourse.mybir as mybir
import concourse.tile as tile
from concourse import bacc
from concourse.bass import ds, ts
from concourse.bass_utils import run_bass_kernel_spmd

B, S, D, H, DH = 2, 2048, 1024, 16, 64
N_CORES = 8
HPC = 4  # heads per core
DSL = HPC * DH  # 256, d_model slice per core
BF16 = mybir.dt.bfloat16
F32 = mybir.dt.float32

QC = 512  # q chunk for score tiles
KT = 128  # k tile (score-tile partition dim)
NQT = S // 128  # 16 q tiles of 128
NQC = S // QC  # 4 q chunks
NKC = D // 128  # 8 contraction chunks for projections
JPQ = QC // KT  # 4 k-tiles (and q-subtiles) per q chunk


def build_nc():
    nc = bacc.Bacc(
        "TRN2",
        target_bir_lowering=False,
        debug=False,
        enable_asserts=False,
        num_devices=N_CORES,
    )
    # host-prearranged layouts (p = partition):
    # xf[p, kc, q]        = x[b].T[kc*128+p, q], q in [0, 512)
    # xr[p, s, kc, q]     = x[b].T[kc*128+p, (s+1)*512 + q]
    # wq/wk[p, ci, kc, n] = W[kc*128+p, ci*128+n]
    # wv[p, kc, n]        = Wv[kc*128+p, n]
    # wo[p, c, n]         = Wo[c*128+p, n]
    xf_ext = nc.dram_tensor("xf", [128, NKC, QC], BF16, kind="ExternalInput")
    xr_ext = nc.dram_tensor("xr", [128, NQC - 1, NKC, QC], BF16, kind="ExternalInput")
    wq_ext = nc.dram_tensor("wq", [128, 2, NKC, 128], BF16, kind="ExternalInput")
    wk_ext = nc.dram_tensor("wk", [128, 2, NKC, 128], BF16, kind="ExternalInput")
    wv_ext = nc.dram_tensor("wv", [128, NKC, DSL], BF16, kind="ExternalInput")
    wo_ext = nc.dram_tensor("wo", [128, 2, D], BF16, kind="ExternalInput")
    bqk_ext = nc.dram_tensor("bqk", [128, 4], F32, kind="ExternalInput")
    out_ext = nc.dram_tensor("out", [S, D], BF16, kind="ExternalOutput")

    with tile.TileContext(nc) as tc:
        with (
            tc.tile_pool(name="consts", bufs=1) as consts,
            tc.tile_pool(name="weights", bufs=1) as weights,
            tc.tile_pool(name="xt", bufs=1) as xt_pool,
            tc.tile_pool(name="qkv", bufs=1) as qkv_pool,
            tc.tile_pool(name="pt", bufs=24) as pt_pool,
            tc.tile_pool(name="norm", bufs=8) as norm_pool,
            tc.tile_pool(name="out_sb", bufs=4) as out_pool,
            # PSUM: 3 x 2-bank (scores/proj) + 2 x 1-bank (P@V, out-proj)
            tc.tile_pool(name="s_psum", bufs=3, space="PSUM") as s_psum,
            tc.tile_pool(name="o_psum", bufs=2, space="PSUM") as o_psum,
        ):
            bqk_sb = consts.tile([128, 4], F32, name="bqk_sb")
            scratch = consts.tile([128, QC], BF16, name="scratch")
            wq_sb = weights.tile([128, 2, NKC, 128], BF16, name="wq_sb")
            wk_sb = weights.tile([128, 2, NKC, 128], BF16, name="wk_sb")
            wv_sb = weights.tile([128, NKC, DSL], BF16, name="wv_sb")
            wo_sb = weights.tile([128, 2, D], BF16, name="wo_sb")
            xf_sb = xt_pool.tile([128, NKC, QC], BF16, name="xf")
            xr_sb = xt_pool.tile([128, NQC - 1, NKC, QC], BF16, name="xr")

            # Three rings, pieces in first-needed order.  The first real
            # matmul chain needs wq ci0 + the xf quarters, so those are
            # spread across all three rings (sync, scalar HWDGE; gpsimd
            # SWDGE) to land ~4-6us in at the shared ~360GB/s HBM ceiling.
            nc.scalar.dma_start(wq_sb[:, 0], wq_ext.ap()[:, 0])
            nc.scalar.dma_start(wk_sb[:, 0], wk_ext.ap()[:, 0])
            nc.scalar.dma_start(wq_sb[:, 1], wq_ext.ap()[:, 1])
            nc.scalar.dma_start(wk_sb[:, 1], wk_ext.ap()[:, 1])
            nc.scalar.dma_start(wo_sb[:], wo_ext.ap())
            nc.sync.dma_start(xf_sb[:, 0:2, :], xf_ext.ap()[:, 0:2, :])
            nc.sync.dma_start(xf_sb[:, 2:4, :], xf_ext.ap()[:, 2:4, :])
            nc.sync.dma_start(xf_sb[:, 4:6, :], xf_ext.ap()[:, 4:6, :])
            nc.gpsimd.dma_start(bqk_sb[:], bqk_ext.ap())
            nc.gpsimd.dma_start(xf_sb[:, 6:8, :], xf_ext.ap()[:, 6:8, :])
            nc.gpsimd.dma_start(wv_sb[:], wv_ext.ap())
            for sc in range(NQC - 1):
                nc.sync.dma_start(
                    xr_sb[:, sc, 0:4], xr_ext.ap()[:, sc, 0:4]
                )
                nc.gpsimd.dma_start(
                    xr_sb[:, sc, 4:8], xr_ext.ap()[:, sc, 4:8]
                )

            bq_sb = bqk_sb[:, 0:2]
            bk_sb = bqk_sb[:, 2:4]

            def xt(kc, sc):
                if sc == 0:
                    return xf_sb[:, kc, :]
                return xr_sb[:, sc - 1, kc, :]

            # --- projection / attention tiles ---
            qt_sb = [qkv_pool.tile([128, S], BF16, name=f"qt{c}") for c in range(2)]
            kt_sb = [qkv_pool.tile([128, S], BF16, name=f"kt{c}") for c in range(2)]
            v_sb = [
                qkv_pool.tile([128, HPC, DH + 1], BF16, name=f"v{st}")
                for st in range(NQT)
            ]
            attnT_sb = [qkv_pool.tile([128, S], BF16, name=f"att{c}") for c in range(2)]

            # --- HAM warm-up: zero scratch, then a burst of dummy matmuls
            # with no DMA dependency so the PE is busy (and un-throttled to
            # 2.4GHz) by the time the first input pieces land. ---
            nc.vector.memset(scratch[:], 0.0)
            dummy_ps = s_psum.tile([128, 2 * QC], F32, name="sc")
            for i in range(22):
                nc.tensor.matmul(
                    dummy_ps[:, ts(i % 2, QC)],
                    scratch[:, 0:128],
                    scratch[:, 0:QC],
                    start=True,
                    stop=True,
                )
            for i in range(6):
                nc.tensor.matmul(
                    dummy_ps[:, ds(128 * (i % 2), 128)],
                    scratch[:, 0:128],
                    scratch[:, 0:128],
                    start=True,
                    stop=True,
                )

            def proj_chunks(sc):
                """Emit-callbacks projecting Q^T/K^T columns and V s-tiles of
                q-chunk sc, one 8-matmul chain each.  For chunk 0 the kc
                contraction follows the DMA arrival order of the xf pieces
                (sync: kc 0-3, gpsimd: kc 6-7 early, scalar: kc 4-5 last)
                so the chain never stalls mid-accumulation; accumulation
                order is irrelevant to the result."""
                kc_order = [0, 1, 6, 7, 2, 3, 4, 5] if sc == 0 else list(range(NKC))

                def qk(dst, w_sb, b_sb, ci):
                    def emit():
                        ps = s_psum.tile([128, 2 * QC], F32, name="sc")
                        for i, kc in enumerate(kc_order):
                            nc.tensor.matmul(
                                ps[:, 0:QC],
                                w_sb[:, ci, kc, :],
                                xt(kc, sc),
                                start=(i == 0),
                                stop=(i == NKC - 1),
                            )
                        nc.vector.tensor_scalar_add(
                            out=dst[ci][:, ts(sc, QC)],
                            in0=ps[:, 0:QC],
                            scalar1=b_sb[:, ds(ci, 1)],
                        )

                    return emit

                def vproj(st):
                    def emit():
                        ps = s_psum.tile([128, 2 * QC], F32, name="sc")
                        for i, kc in enumerate(kc_order):
                            nc.tensor.matmul(
                                ps[:, 0:DSL],
                                xt(kc, st // JPQ)[:, ts(st % JPQ, 128)],
                                wv_sb[:, kc, :],
                                start=(i == 0),
                                stop=(i == NKC - 1),
                            )
                        nc.vector.tensor_copy(
                            v_sb[st][:, :, 0:DH],
                            ps[:, 0:DSL].rearrange("p (h d) -> p h d", h=HPC),
                        )
                        nc.gpsimd.memset(v_sb[st][:, :, DH : DH + 1], 1.0)

                    return emit

                chains = []
                for ci in range(2):
                    chains.append(qk(qt_sb, wq_sb, bq_sb, ci))
                    chains.append(qk(kt_sb, wk_sb, bk_sb, ci))
                for st in range(JPQ * sc, JPQ * (sc + 1)):
                    chains.append(vproj(st))
                return chains

            def tile_layout(qc):
                n_kt = (qc + 1) * JPQ
                width = [QC - KT * max(0, kt - qc * JPQ) for kt in range(n_kt)]
                off = [0 if kt % 2 == 0 else width[kt - 1] for kt in range(n_kt)]
                return n_kt, width, off

            def mask_tiles(qc, pt, g):
                n_kt, width, off = tile_layout(qc)
                for t in range(2):
                    kt = g * 2 + t
                    if width[kt] < QC or kt == qc * JPQ:
                        # causal mask: zero where p > f (gpsimd, off the
                        # DVE critical path)
                        nc.gpsimd.affine_select(
                            out=pt[:, ds(off[kt], width[kt])],
                            in_=pt[:, ds(off[kt], width[kt])],
                            compare_op=mybir.AluOpType.is_ge,
                            fill=0.0,
                            base=0,
                            pattern=[[1, width[kt]]],
                            channel_multiplier=-1,
                        )

            def scores_pair_kt(qc, ci, kt, pt_lo, pt_hi):
                """Score tile kt for BOTH heads of column-half ci, emitted as
                adjacent K=64 matmuls on complementary partition ranges
                (rows 0-63 / 64-127) so the PE runs them concurrently via
                row tiling.  Both heads land in ONE wide psum tile (lo at
                col 0, hi at col `hi_off`) so a SINGLE exp covers both,
                halving the scalar per-call overhead.  hi_off is `width`
                when both halves fit in one 512-col psum bank, else 512
                (the <=128 garbage cols in between get exp'd and ignored)."""
                n_kt, width, off = tile_layout(qc)
                w = width[kt]
                qoff = qc * QC + (QC - w)
                ps = s_psum.tile([128, 2 * QC], F32, name="sc")
                pt = pt_pool.tile([128, 2 * QC], BF16, name="pt")
                for hh in range(2):
                    po = hh * 64
                    # hi head always at col QC: a different psum bank, so
                    # the two concurrent row-tiled matmuls never write the
                    # same bank.
                    nc.tensor.matmul(
                        ps[:, ds(hh * QC, w)],
                        kt_sb[ci][po : po + 64, ts(kt, KT)],
                        qt_sb[ci][po : po + 64, ds(qoff, w)],
                        start=True,
                        stop=True,
                    )
                # one exp covers both heads via a 2-region strided AP
                nc.scalar.activation(
                    pt.rearrange("p (two q) -> p two q", two=2)[:, :, 0:w],
                    ps.rearrange("p (two q) -> p two q", two=2)[:, :, 0:w],
                    mybir.ActivationFunctionType.Exp,
                    scale=0.125,
                )
                lo_view = pt[:, ds(0, w)]
                hi_view = pt[:, ds(QC, w)]
                if w < QC or kt == qc * JPQ:
                    for view in (lo_view, hi_view):
                        nc.gpsimd.affine_select(
                            out=view,
                            in_=view,
                            compare_op=mybir.AluOpType.is_ge,
                            fill=0.0,
                            base=0,
                            pattern=[[1, w]],
                            channel_multiplier=-1,
                        )
                pt_lo.append(lo_view)
                pt_hi.append(hi_view)

            def scores_group(qc, h, pt_g, g):
                ci, po = divmod(h, 2)
                po *= 64
                n_kt, width, off = tile_layout(qc)
                used = width[2 * g] + width[2 * g + 1]
                ps = s_psum.tile([128, 2 * QC], F32, name="sc")
                pt = pt_pool.tile([128, 2 * QC], BF16, name="pt")
                for t in range(2):
                    kt = g * 2 + t
                    qoff = qc * QC + (QC - width[kt])
                    nc.tensor.matmul(
                        ps[:, ds(off[kt], width[kt])],
                        kt_sb[ci][po : po + 64, ts(kt, KT)],
                        qt_sb[ci][po : po + 64, ds(qoff, width[kt])],
                        start=True,
                        stop=True,
                    )
                nc.scalar.activation(
                    pt[:, 0:used],
                    ps[:, 0:used],
                    mybir.ActivationFunctionType.Exp,
                    scale=0.125,
                )
                mask_tiles(qc, pt, g)
                for t in range(2):
                    kt = g * 2 + t
                    pt_g.append(pt[:, ds(off[kt], width[kt])])

            def scores(qc, h, pt_g):
                n_kt, _, _ = tile_layout(qc)
                for g in range(n_kt // 2):
                    scores_group(qc, h, pt_g, g)

            def pv_chain(qc, h, pt_g, po_, kts):
                n_kt, width, off = tile_layout(qc)
                for kt in kts:
                    nc.tensor.matmul(
                        po_[0 : DH + 1, ds(QC - width[kt], width[kt])],
                        v_sb[kt][:, h, :],
                        pt_g[kt],
                        start=(kt == 0),
                        stop=(kt == n_kt - 1),
                    )

            def pv_chain_slice(qc, h, pt_g, po_, kts, lo, hi, stop_kt):
                """P@V partial chain restricted to chunk columns [lo, hi)."""
                n_kt, width, off = tile_layout(qc)
                for kt in kts:
                    qoff = QC - width[kt]  # first chunk column this k-tile covers
                    a = max(lo, qoff)
                    if a >= hi:
                        continue
                    nc.tensor.matmul(
                        po_[0 : DH + 1, ds(a, hi - a)],
                        v_sb[kt][:, h, :],
                        pt_g[kt][:, ds(a - qoff, hi - a)],
                        start=(kt == 0),
                        stop=(kt == stop_kt),
                    )

            def pv_norm(qc, h, pt_g, po_=None, skip_chain=False):
                ci, po = divmod(h, 2)
                po *= 64
                n_kt, width, off = tile_layout(qc)
                if po_ is None:
                    po_ = o_psum.tile([128, QC], F32, name="ov")
                if not skip_chain:
                    pv_chain(qc, h, pt_g, po_, range(n_kt))
                den = norm_pool.tile([64, QC], F32, name="den")
                row = norm_pool.tile([1, QC], F32, name="row")
                nc.vector.tensor_copy(row[:], po_[DH : DH + 1, :])
                nc.vector.reciprocal_approx_fast(den[0:1, :], row[:])
                nc.gpsimd.partition_broadcast(den[:], den[0:1, :])
                nc.vector.tensor_mul(
                    attnT_sb[ci][po : po + 64, ts(qc, QC)],
                    po_[0:DH, :],
                    den[:],
                )

            def out_proj_tile(qc, j, pool=None, tail=False):
                qt = qc * JPQ + j
                o_sb = out_pool.tile([128, D], BF16, name="osb")
                for ncol in range(2):
                    if pool is None:
                        pu = o_psum.tile([128, QC], F32, name="ov")
                    else:
                        # final chunk: the scores pool is idle by now; use
                        # its 3 wide slots so all 4 tail tiles overlap
                        pu = pool.tile([128, 2 * QC], F32, name="sc")[:, 0:QC]
                    for ci in range(2):
                        nc.tensor.matmul(
                            pu[:],
                            attnT_sb[ci][:, ts(qt, 128)],
                            wo_sb[:, ci, ts(ncol, 512)],
                            start=(ci == 0),
                            stop=(ci == 1),
                        )
                    # tail tiles: second half cast via scalar-engine Copy
                    # activation (same act table as Exp; scalar is idle in
                    # the tail) so the two casts run in parallel and the
                    # DMA issues sooner. gpsimd can't read PSUM.
                    if tail and ncol == 1:
                        nc.scalar.activation(
                            o_sb[:, ts(ncol, 512)],
                            pu[:],
                            mybir.ActivationFunctionType.Copy,
                        )
                    else:
                        nc.vector.tensor_copy(o_sb[:, ts(ncol, 512)], pu[:])
                ring = nc.sync if qt % 2 == 0 else nc.gpsimd
                ring.dma_start(out_ext.ap()[ts(qt, 128), :], o_sb[:])

            # ---------------- emission schedule ----------------
            # Engines execute strictly in program order, so PE bubbles in the
            # exp-latency-bound attention pipeline must be filled by
            # interleaving independent matmul work (projection chains for
            # q-chunk qc+1 and the previous chunk's out-projection) at
            # emission granularity.  The slot layout matches the original
            # per-head schedule; for qc 0-2 each per-head scores slot holds
            # half of that ci's PAIRED groups instead (both heads at once),
            # so exp/mask pacing per slot is unchanged while the PE time
            # halves.  qc3 stays unpaired: it is scalar(exp)-bound, so
            # pairing would only move the wait.
            c0 = proj_chunks(0)  # [Q0, K0, Q1, K1, V0..V3]
            pt_g0 = [[] for _ in range(HPC)]
            c0[0]()
            c0[1]()
            for kt in (0, 1):
                scores_pair_kt(0, 0, kt, pt_g0[0], pt_g0[1])
            c0[2]()
            c0[3]()
            for kt in (2, 3):
                scores_pair_kt(0, 0, kt, pt_g0[0], pt_g0[1])
            for emit in c0[4:]:
                emit()
            for qc in range(NQC):
                filler = proj_chunks(qc + 1) if qc + 1 < NQC else []
                fi = 0

                def fill(n):
                    nonlocal fi
                    for _ in range(n):
                        if fi < len(filler):
                            filler[fi]()
                            fi += 1

                def oprev(j):
                    if qc > 0:
                        out_proj_tile(qc - 1, j)

                pt_gs = pt_g0 if qc == 0 else [[] for _ in range(HPC)]
                ng = (qc + 1) * JPQ // 2
                paired = qc < NQC - 1
                if qc > 0:
                    # ci0 is paired for every chunk: per-kt pairing leaves
                    # the scalar exp data/call count unchanged, so even the
                    # exp-heavy final chunk nets the halved PE score time.
                    n_kt_qc = (qc + 1) * JPQ
                    for kt in range(n_kt_qc // 2):
                        scores_pair_kt(qc, 0, kt, pt_gs[0], pt_gs[1])
                    fill(2)
                    oprev(0)
                    for kt in range(n_kt_qc // 2, n_kt_qc):
                        scores_pair_kt(qc, 0, kt, pt_gs[0], pt_gs[1])
                    fill(1)
                else:
                    fill(3)
                pv_norm(qc, 0, pt_gs[0])
                fill(2)
                oprev(1)
                if paired:
                    n_kt_qc = (qc + 1) * JPQ
                    for kt in range(n_kt_qc // 2):
                        scores_pair_kt(qc, 1, kt, pt_gs[2], pt_gs[3])
                else:
                    scores(qc, 2, pt_gs[2])
                fill(1)
                pv_norm(qc, 1, pt_gs[1])
                fill(2)
                oprev(2)
                if qc == NQC - 1:
                    # tail: head 3's P@V runs as two column-slice chains (A =
                    # chunk cols [0,256), B = [256,512)) both pipelined into
                    # the score-group loop, so every PSUM write lands shortly
                    # after the last exp; head 2's full chain fills the
                    # remaining exp-latency bubbles. oprev(3) is emitted
                    # BEFORE po2/po3 so its o_psum slots pair with
                    # earlier (fully-read) tiles (slot-reuse WAR order).
                    oprev(3)
                    n_kt = (qc + 1) * JPQ
                    po2 = o_psum.tile([128, QC], F32, name="ov")
                    po3 = o_psum.tile([128, QC], F32, name="ov")
                    for g in range(n_kt // 2):
                        scores_group(qc, 3, pt_gs[3], g)
                        pv_chain(qc, 2, pt_gs[2], po2, [2 * g, 2 * g + 1])
                        if g >= 3:
                            pv_chain_slice(
                                qc, 3, pt_gs[3], po3,
                                [2 * g - 6, 2 * g - 5], 0, 256, n_kt - 3,
                            )
                    pv_chain_slice(
                        qc, 3, pt_gs[3], po3, range(10, n_kt - 2), 0, 256,
                        n_kt - 3,
                    )
                    # slice B needs its own PSUM bank: 'start' resets the
                    # whole bank's accumulation, so a second chain cannot
                    # share po3 with the (already accumulating) slice A.
                    # An s_psum tile allocated here pairs, in slot rotation,
                    # with a long-retired scores group.
                    ps_b = s_psum.tile([128, 2 * QC], F32, name="sc")[:, 0:QC]
                    pv_chain_slice(
                        qc, 3, pt_gs[3], ps_b, range(n_kt), 256, QC,
                        n_kt - 1,
                    )
                    # heads 2+3 normalize, emitted copies/recips first and
                    # muls last so the serial gpsimd broadcast chain overlaps
                    # the vector work and the PE's out-proj stream; head 3 is
                    # split in halves so its first mul lands sooner
                    den2 = norm_pool.tile([64, QC], F32, name="den")
                    row2 = norm_pool.tile([1, QC], F32, name="row")
                    den3 = norm_pool.tile([64, QC], F32, name="den")
                    row3 = norm_pool.tile([1, QC], F32, name="row")
                    nc.vector.tensor_copy(row2[:], po2[DH : DH + 1, :])
                    nc.vector.reciprocal_approx_fast(den2[0:1, :], row2[:])
                    nc.vector.tensor_copy(row3[:, 0:256], po3[DH : DH + 1, 0:256])
                    nc.vector.tensor_copy(
                        row3[:, 256:QC], ps_b[DH : DH + 1, 256:QC]
                    )
                    nc.vector.reciprocal_approx_fast(
                        den3[0:1, 0:256], row3[:, 0:256]
                    )
                    nc.vector.reciprocal_approx_fast(
                        den3[0:1, 256:QC], row3[:, 256:QC]
                    )
                    nc.gpsimd.partition_broadcast(den2[:], den2[0:1, :])
                    nc.gpsimd.partition_broadcast(
                        den3[:, 0:256], den3[0:1, 0:256]
                    )
                    nc.gpsimd.partition_broadcast(
                        den3[:, 256:QC], den3[0:1, 256:QC]
                    )
                    nc.vector.tensor_mul(
                        attnT_sb[1][0:64, ts(qc, QC)],
                        po2[0:DH, :],
                        den2[:],
                    )
                    nc.vector.tensor_mul(
                        attnT_sb[1][64:128, ds(qc * QC, 256)],
                        po3[0:DH, 0:256],
                        den3[:, 0:256],
                    )
                    nc.vector.tensor_mul(
                        attnT_sb[1][64:128, ds(qc * QC + 256, 256)],
                        ps_b[0:DH, 256:QC],
                        den3[:, 256:QC],
                    )
                    # tail out-proj in two phases: the ci0 half-contraction
                    # (heads 0/1, normalized long ago) streams right after
                    # chainB so the PE never idles (an idle gap here drops
                    # the PE to a half-rate p-state for the whole drain);
                    # the ci1 half lands as the head-2/3 normalizes complete.
                    # tiles 0-2: one wide s_psum tile each (2 banks = both
                    # ncol halves). tile 3: the two o_psum slots — their
                    # pending readers (the norm ops above) are already
                    # emitted, so slot-reuse WARs can't cycle with the PE.
                    pws = [
                        s_psum.tile([128, 2 * QC], F32, name="sc")
                        for _ in range(3)
                    ]
                    po_t3 = [
                        o_psum.tile([128, QC], F32, name="ov") for _ in range(2)
                    ]

                    def tail_pu(j, ncol):
                        if j < 3:
                            return pws[j][:, ts(ncol, QC)]
                        return po_t3[ncol][:]

                    o_sbs = [
                        out_pool.tile([128, D], BF16, name="osb")
                        for _ in range(JPQ)
                    ]
                    for j in range(JPQ):
                        qt = qc * JPQ + j
                        for ncol in range(2):
                            nc.tensor.matmul(
                                tail_pu(j, ncol),
                                attnT_sb[0][:, ts(qt, 128)],
                                wo_sb[:, 0, ts(ncol, 512)],
                                start=True,
                                stop=False,
                            )
                    for j in range(JPQ):
                        qt = qc * JPQ + j
                        for ncol in range(2):
                            pu = tail_pu(j, ncol)
                            nc.tensor.matmul(
                                pu,
                                attnT_sb[1][:, ts(qt, 128)],
                                wo_sb[:, 1, ts(ncol, 512)],
                                start=False,
                                stop=True,
                            )
                            if ncol == 1:
                                nc.scalar.activation(
                                    o_sbs[j][:, ts(ncol, 512)],
                                    pu,
                                    mybir.ActivationFunctionType.Copy,
                                )
                            else:
                                nc.vector.tensor_copy(
                                    o_sbs[j][:, ts(ncol, 512)], pu
                                )
                        ring = nc.sync if qt % 2 == 0 else nc.gpsimd
                        ring.dma_start(
                            out_ext.ap()[ts(qt, 128), :], o_sbs[j][:]
                        )
                else:
                    if paired:
                        n_kt_qc = (qc + 1) * JPQ
                        for kt in range(n_kt_qc // 2, n_kt_qc):
                            scores_pair_kt(qc, 1, kt, pt_gs[2], pt_gs[3])
                    else:
                        scores(qc, 3, pt_gs[3])
                    fill(1)
                    pv_norm(qc, 2, pt_gs[2])
                    fill(2)
                    oprev(3)
                    pv_norm(qc, 3, pt_gs[3])
                    fill(len(filler) - fi)

    nc.compile()
    return nc


_NC_CACHE = None


def _get_nc():
    global _NC_CACHE
    if _NC_CACHE is None:
        _NC_CACHE = build_nc()
    return _NC_CACHE


def make_in_maps(x, Wq, bq, Wk, bk, Wv, bv, Wo, bo):
    bf = ml_dtypes.bfloat16

    def qk_chunked(w):  # [1024, 256] -> [128, 2, 8, 128]
        return np.ascontiguousarray(
            w.reshape(NKC, 128, 2, 128).transpose(1, 2, 0, 3)
        ).astype(bf)

    in_maps = []
    for c in range(N_CORES):
        b, g = c // HPC, c % HPC
        lo, hi = g * DSL, (g + 1) * DSL
        xT = np.asarray(x[b]).T  # [1024, 2048]
        xf = xT[:, :QC].reshape(NKC, 128, QC).transpose(1, 0, 2)
        xr = (
            xT[:, QC:]
            .reshape(NKC, 128, NQC - 1, QC)
            .transpose(1, 2, 0, 3)  # [p, sc-1, kc, q]
        )
        bqk = np.stack(
            [bq[lo : lo + 128], bq[lo + 128 : hi], bk[lo : lo + 128],
             bk[lo + 128 : hi]],
            axis=1,
        )
        in_maps.append(
            {
                "xf": np.ascontiguousarray(xf).astype(bf),
                "xr": np.ascontiguousarray(xr).astype(bf),
                "wq": qk_chunked(Wq[:, lo:hi]),
                "wk": qk_chunked(Wk[:, lo:hi]),
                "wv": np.ascontiguousarray(
                    Wv[:, lo:hi].reshape(NKC, 128, DSL).transpose(1, 0, 2)
                ).astype(bf),
                "wo": np.ascontiguousarray(
                    Wo[lo:hi, :].reshape(2, 128, D).transpose(1, 0, 2)
                ).astype(bf),
                "bqk": np.ascontiguousarray(bqk).astype(np.float32),
            }
        )
    return in_maps


def gather_output(results, bv, Wo, bo):
    # softmax rows sum to 1, so the V-bias contributes bv @ Wo to every row
    corr = (np.asarray(bv, np.float64) @ np.asarray(Wo, np.float64)).astype(
        np.float32
    ) + np.asarray(bo, np.float32)
    out = np.empty((B, S, D), np.float32)
    for b in range(B):
        acc = np.zeros((S, D), np.float32)
        for g in range(HPC):
            acc += results[b * HPC + g]["out"].astype(np.float32)
        out[b] = acc + corr
    return out


def kernel(x, Wq, bq, Wk, bk, Wv, bv, Wo, bo, _trace=False):
    x = np.asarray(x, np.float32)
    nc = _get_nc()
    in_maps = make_in_maps(x, Wq, bq, Wk, bk, Wv, bv, Wo, bo)
    res = run_bass_kernel_spmd(nc, in_maps, list(range(N_CORES)), trace=_trace)
    out = gather_output(res.results, bv, Wo, bo)
    if _trace:
        return out, res
    return out


# revision 18
# speedup vs baseline: 1.0094x; 1.0094x over previous
"""Trainium2 Bass kernel for nn_CausalAttention (B=2, S=2048, D=1024, H=16).

Sharding: tensor-parallel over heads (4 groups of 4 heads) x data-parallel
over batch (2), on 8 NeuronCores. Core c handles batch b = c // 4 and head
group g = c % 4 (heads 4g..4g+3, i.e. d_model columns 256g..256g+256).

Each core computes, fully on-device in bf16 (f32 PSUM accumulation):
  Q^T, K^T (d_out on partitions) and V (s on partitions, ones column
  appended) for its head slice, projected incrementally per q-chunk so
  projection matmuls overlap the (exp-latency-bound) attention pipeline;
  transposed score tiles (k on partitions, q free) trimmed to the causal
  region; P^T = exp(S^T / 8) with causal masking on GPSIMD (no max
  subtraction -- scores are O(1) here); unnormalized attention out^T[dh, q]
  via V-stationary matmuls where the ones column yields the softmax
  denominator for free; normalization by the broadcast fast-approx
  reciprocal of the denominator row; then a partial out = attn @ Wo_slice,
  interleaved per q-chunk.

Score matmuls contract over dh=64 only, so the two heads of a W-column
half (partitions 0-63 / 64-127) are emitted as ADJACENT matmul pairs:
their auto-derived PE tile_positions (rows 0-63 vs 64-127) let the two
K=64 matmuls run concurrently in the PE array (row tiling), halving the
score-phase PE time. A zero-filled scratch feeds a ~9us burst of dummy
matmuls at program start so the PE is continuously busy from ~8us on and
the HAM clock-gate releases (2.4 GHz) before the first real chains run --
with no >3.4us PE gap afterwards the clock never re-throttles mid-run.
The first-needed input pieces (wq/wk ci0, the xf quarters) are spread
across all three DMA rings, and chunk-0 projection chains accumulate kc
in DMA-arrival order (sync: 0-3, gpsimd: 6-7, scalar-ring xf45 last) so
the first chains never stall mid-accumulation.

DMA strategy: inputs are pre-arranged host-side so every DMA has 2-4KB
contiguous per-partition lines, split into ~0.25-0.5MB pieces issued in
first-needed order across the three DMA rings (sync, scalar, gpsimd).
The output is written in bf16 (host accumulates partials in f32), one DMA
per 128-row tile, alternating sync/gpsimd rings.

Tail: the final q-chunk's last head splits its P@V into two column-slice
chains, both pipelined into the score-group loop so every PSUM write
lands ~1.5us after the last exp; the three remaining normalizes are
emitted copies/recips-first so the gpsimd broadcasts chain while the PE
streams the out-projection tiles, whose psum->sbuf bf16 casts are split
between the vector and gpsimd engines.

Host-side glue (sharding/gather): x is pre-transposed per batch, weights are
pre-sliced and cast to bf16; the 4 partial outputs per batch are summed and
bo + bv @ Wo (the V-bias contribution, exact since softmax rows sum to 1)
is added.
"""

import sys

for _p in ("/opt/trn_rl_repo",):
    if _p not in sys.path:
        sys.path.append(_p)

import ml_dtypes
import numpy as np

import concourse.bass as bass
import concourse.mybir as mybir
import concourse.tile as tile
from concourse import bacc
from concourse.bass import ds, ts
from concourse.bass_utils import run_bass_kernel_spmd

B, S, D, H, DH = 2, 2048, 1024, 16, 64
N_CORES = 8
HPC = 4  # heads per core
DSL = HPC * DH  # 256, d_model slice per core
BF16 = mybir.dt.bfloat16
F32 = mybir.dt.float32

QC = 512  # q chunk for score tiles
KT = 128  # k tile (score-tile partition dim)
NQT = S // 128  # 16 q tiles of 128
NQC = S // QC  # 4 q chunks
NKC = D // 128  # 8 contraction chunks for projections
JPQ = QC // KT  # 4 k-tiles (and q-subtiles) per q chunk


def build_nc():
    nc = bacc.Bacc(
        "TRN2",
        target_bir_lowering=False,
        debug=False,
        enable_asserts=False,
        num_devices=N_CORES,
    )
    # host-prearranged layouts (p = partition):
    # xf[p, kc, q]        = x[b].T[kc*128+p, q], q in [0, 512)
    # xr[p, s, kc, q]     = x[b].T[kc*128+p, (s+1)*512 + q]
    # wq/wk[p, ci, kc, n] = W[kc*128+p, ci*128+n]
    # wv[p, kc, n]        = Wv[kc*128+p, n]
    # wo[p, c, n]         = Wo[c*128+p, n]
    xf_ext = nc.dram_tensor("xf", [128, NKC, QC], BF16, kind="ExternalInput")
    xr_ext = nc.dram_tensor("xr", [128, NQC - 1, NKC, QC], BF16, kind="ExternalInput")
    wq_ext = nc.dram_tensor("wq", [128, 2, NKC, 128], BF16, kind="ExternalInput")
    wk_ext = nc.dram_tensor("wk", [128, 2, NKC, 128], BF16, kind="ExternalInput")
    wv_ext = nc.dram_tensor("wv", [128, NKC, DSL], BF16, kind="ExternalInput")
    wo_ext = nc.dram_tensor("wo", [128, 2, D], BF16, kind="ExternalInput")
    bqk_ext = nc.dram_tensor("bqk", [128, 4], F32, kind="ExternalInput")
    out_ext = nc.dram_tensor("out", [S, D], BF16, kind="ExternalOutput")

    with tile.TileContext(nc) as tc:
        with (
            tc.tile_pool(name="consts", bufs=1) as consts,
            tc.tile_pool(name="weights", bufs=1) as weights,
            tc.tile_pool(name="xt", bufs=1) as xt_pool,
            tc.tile_pool(name="qkv", bufs=1) as qkv_pool,
            tc.tile_pool(name="pt", bufs=24) as pt_pool,
            tc.tile_pool(name="norm", bufs=8) as norm_pool,
            tc.tile_pool(name="out_sb", bufs=4) as out_pool,
            # PSUM: 3 x 2-bank (scores/proj) + 2 x 1-bank (P@V, out-proj)
            tc.tile_pool(name="s_psum", bufs=3, space="PSUM") as s_psum,
            tc.tile_pool(name="o_psum", bufs=2, space="PSUM") as o_psum,
        ):
            bqk_sb = consts.tile([128, 4], F32, name="bqk_sb")
            scratch = consts.tile([128, QC], BF16, name="scratch")
            wq_sb = weights.tile([128, 2, NKC, 128], BF16, name="wq_sb")
            wk_sb = weights.tile([128, 2, NKC, 128], BF16, name="wk_sb")
            wv_sb = weights.tile([128, NKC, DSL], BF16, name="wv_sb")
            wo_sb = weights.tile([128, 2, D], BF16, name="wo_sb")
            xf_sb = xt_pool.tile([128, NKC, QC], BF16, name="xf")
            xr_sb = xt_pool.tile([128, NQC - 1, NKC, QC], BF16, name="xr")

            # Three rings, pieces in first-needed order.  The first real
            # matmul chain needs wq ci0 + the xf quarters, so those are
            # spread across all three rings (sync, scalar HWDGE; gpsimd
            # SWDGE) to land ~4-6us in at the shared ~360GB/s HBM ceiling.
            nc.scalar.dma_start(wq_sb[:, 0], wq_ext.ap()[:, 0])
            nc.scalar.dma_start(wk_sb[:, 0], wk_ext.ap()[:, 0])
            nc.scalar.dma_start(wq_sb[:, 1], wq_ext.ap()[:, 1])
            nc.scalar.dma_start(wk_sb[:, 1], wk_ext.ap()[:, 1])
            nc.scalar.dma_start(wo_sb[:], wo_ext.ap())
            nc.sync.dma_start(xf_sb[:, 0:2, :], xf_ext.ap()[:, 0:2, :])
            nc.sync.dma_start(xf_sb[:, 2:4, :], xf_ext.ap()[:, 2:4, :])
            nc.sync.dma_start(xf_sb[:, 4:6, :], xf_ext.ap()[:, 4:6, :])
            nc.gpsimd.dma_start(bqk_sb[:], bqk_ext.ap())
            nc.gpsimd.dma_start(xf_sb[:, 6:8, :], xf_ext.ap()[:, 6:8, :])
            nc.gpsimd.dma_start(wv_sb[:], wv_ext.ap())
            for sc in range(NQC - 1):
                nc.sync.dma_start(
                    xr_sb[:, sc, 0:4], xr_ext.ap()[:, sc, 0:4]
                )
                nc.gpsimd.dma_start(
                    xr_sb[:, sc, 4:8], xr_ext.ap()[:, sc, 4:8]
                )

            bq_sb = bqk_sb[:, 0:2]
            bk_sb = bqk_sb[:, 2:4]

            def xt(kc, sc):
                if sc == 0:
                    return xf_sb[:, kc, :]
                return xr_sb[:, sc - 1, kc, :]

            # --- projection / attention tiles ---
            qt_sb = [qkv_pool.tile([128, S], BF16, name=f"qt{c}") for c in range(2)]
            kt_sb = [qkv_pool.tile([128, S], BF16, name=f"kt{c}") for c in range(2)]
            v_sb = [
                qkv_pool.tile([128, HPC, DH + 1], BF16, name=f"v{st}")
                for st in range(NQT)
            ]
            attnT_sb = [qkv_pool.tile([128, S], BF16, name=f"att{c}") for c in range(2)]

            # --- HAM warm-up: zero scratch, then a burst of dummy matmuls
            # with no DMA dependency so the PE is busy (and un-throttled to
            # 2.4GHz) by the time the first input pieces land. ---
            nc.vector.memset(scratch[:], 0.0)
            dummy_ps = s_psum.tile([128, 2 * QC], F32, name="sc")
            for i in range(22):
                nc.tensor.matmul(
                    dummy_ps[:, ts(i % 2, QC)],
                    scratch[:, 0:128],
                    scratch[:, 0:QC],
                    start=True,
                    stop=True,
                )
            for i in range(6):
                nc.tensor.matmul(
                    dummy_ps[:, ds(128 * (i % 2), 128)],
                    scratch[:, 0:128],
                    scratch[:, 0:128],
                    start=True,
                    stop=True,
                )

            def proj_chunks(sc):
                """Emit-callbacks projecting Q^T/K^T columns and V s-tiles of
                q-chunk sc, one 8-matmul chain each.  For chunk 0 the kc
                contraction follows the DMA arrival order of the xf pieces
                (sync: kc 0-3, gpsimd: kc 6-7 early, scalar: kc 4-5 last)
                so the chain never stalls mid-accumulation; accumulation
                order is irrelevant to the result."""
                kc_order = [0, 1, 6, 7, 2, 3, 4, 5] if sc == 0 else list(range(NKC))

                def qk(dst, w_sb, b_sb, ci):
                    def emit():
                        ps = s_psum.tile([128, 2 * QC], F32, name="sc")
                        for i, kc in enumerate(kc_order):
                            nc.tensor.matmul(
                                ps[:, 0:QC],
                                w_sb[:, ci, kc, :],
                                xt(kc, sc),
                                start=(i == 0),
                                stop=(i == NKC - 1),
                            )
                        nc.vector.tensor_scalar_add(
                            out=dst[ci][:, ts(sc, QC)],
                            in0=ps[:, 0:QC],
                            scalar1=b_sb[:, ds(ci, 1)],
                        )

                    return emit

                def vproj(st):
                    def emit():
                        ps = s_psum.tile([128, 2 * QC], F32, name="sc")
                        for i, kc in enumerate(kc_order):
                            nc.tensor.matmul(
                                ps[:, 0:DSL],
                                xt(kc, st // JPQ)[:, ts(st % JPQ, 128)],
                                wv_sb[:, kc, :],
                                start=(i == 0),
                                stop=(i == NKC - 1),
                            )
                        nc.vector.tensor_copy(
                            v_sb[st][:, :, 0:DH],
                            ps[:, 0:DSL].rearrange("p (h d) -> p h d", h=HPC),
                        )
                        nc.gpsimd.memset(v_sb[st][:, :, DH : DH + 1], 1.0)

                    return emit

                chains = []
                for ci in range(2):
                    chains.append(qk(qt_sb, wq_sb, bq_sb, ci))
                    chains.append(qk(kt_sb, wk_sb, bk_sb, ci))
                for st in range(JPQ * sc, JPQ * (sc + 1)):
                    chains.append(vproj(st))
                return chains

            def tile_layout(qc):
                n_kt = (qc + 1) * JPQ
                width = [QC - KT * max(0, kt - qc * JPQ) for kt in range(n_kt)]
                off = [0 if kt % 2 == 0 else width[kt - 1] for kt in range(n_kt)]
                return n_kt, width, off

            def mask_tiles(qc, pt, g):
                n_kt, width, off = tile_layout(qc)
                for t in range(2):
                    kt = g * 2 + t
                    if width[kt] < QC or kt == qc * JPQ:
                        # causal mask: zero where p > f (gpsimd, off the
                        # DVE critical path)
                        nc.gpsimd.affine_select(
                            out=pt[:, ds(off[kt], width[kt])],
                            in_=pt[:, ds(off[kt], width[kt])],
                            compare_op=mybir.AluOpType.is_ge,
                            fill=0.0,
                            base=0,
                            pattern=[[1, width[kt]]],
                            channel_multiplier=-1,
                        )

            def scores_pair_kt(qc, ci, kt, pt_lo, pt_hi):
                """Score tile kt for BOTH heads of column-half ci, emitted as
                adjacent K=64 matmuls on complementary partition ranges
                (rows 0-63 / 64-127) so the PE runs them concurrently via
                row tiling.  Both heads land in ONE wide psum tile (lo at
                col 0, hi at col `hi_off`) so a SINGLE exp covers both,
                halving the scalar per-call overhead.  hi_off is `width`
                when both halves fit in one 512-col psum bank, else 512
                (the <=128 garbage cols in between get exp'd and ignored)."""
                n_kt, width, off = tile_layout(qc)
                w = width[kt]
                qoff = qc * QC + (QC - w)
                ps = s_psum.tile([128, 2 * QC], F32, name="sc")
                pt = pt_pool.tile([128, 2 * QC], BF16, name="pt")
                for hh in range(2):
                    po = hh * 64
                    # hi head always at col QC: a different psum bank, so
                    # the two concurrent row-tiled matmuls never write the
                    # same bank.
                    nc.tensor.matmul(
                        ps[:, ds(hh * QC, w)],
                        kt_sb[ci][po : po + 64, ts(kt, KT)],
                        qt_sb[ci][po : po + 64, ds(qoff, w)],
                        start=True,
                        stop=True,
                    )
                # one exp covers both heads via a 2-region strided AP
                nc.scalar.activation(
                    pt.rearrange("p (two q) -> p two q", two=2)[:, :, 0:w],
                    ps.rearrange("p (two q) -> p two q", two=2)[:, :, 0:w],
                    mybir.ActivationFunctionType.Exp,
                    scale=0.125,
                )
                lo_view = pt[:, ds(0, w)]
                hi_view = pt[:, ds(QC, w)]
                if w < QC or kt == qc * JPQ:
                    for view in (lo_view, hi_view):
                        nc.gpsimd.affine_select(
                            out=view,
                            in_=view,
                            compare_op=mybir.AluOpType.is_ge,
                            fill=0.0,
                            base=0,
                            pattern=[[1, w]],
                            channel_multiplier=-1,
                        )
                pt_lo.append(lo_view)
                pt_hi.append(hi_view)

            def scores_group(qc, h, pt_g, g):
                ci, po = divmod(h, 2)
                po *= 64
                n_kt, width, off = tile_layout(qc)
                used = width[2 * g] + width[2 * g + 1]
                ps = s_psum.tile([128, 2 * QC], F32, name="sc")
                pt = pt_pool.tile([128, 2 * QC], BF16, name="pt")
                for t in range(2):
                    kt = g * 2 + t
                    qoff = qc * QC + (QC - width[kt])
                    nc.tensor.matmul(
                        ps[:, ds(off[kt], width[kt])],
                        kt_sb[ci][po : po + 64, ts(kt, KT)],
                        qt_sb[ci][po : po + 64, ds(qoff, width[kt])],
                        start=True,
                        stop=True,
                    )
                nc.scalar.activation(
                    pt[:, 0:used],
                    ps[:, 0:used],
                    mybir.ActivationFunctionType.Exp,
                    scale=0.125,
                )
                mask_tiles(qc, pt, g)
                for t in range(2):
                    kt = g * 2 + t
                    pt_g.append(pt[:, ds(off[kt], width[kt])])

            def scores(qc, h, pt_g):
                n_kt, _, _ = tile_layout(qc)
                for g in range(n_kt // 2):
                    scores_group(qc, h, pt_g, g)

            def pv_chain(qc, h, pt_g, po_, kts):
                n_kt, width, off = tile_layout(qc)
                for kt in kts:
                    nc.tensor.matmul(
                        po_[0 : DH + 1, ds(QC - width[kt], width[kt])],
                        v_sb[kt][:, h, :],
                        pt_g[kt],
                        start=(kt == 0),
                        stop=(kt == n_kt - 1),
                    )

            def pv_chain_slice(qc, h, pt_g, po_, kts, lo, hi, stop_kt):
                """P@V partial chain restricted to chunk columns [lo, hi)."""
                n_kt, width, off = tile_layout(qc)
                for kt in kts:
                    qoff = QC - width[kt]  # first chunk column this k-tile covers
                    a = max(lo, qoff)
                    if a >= hi:
                        continue
                    nc.tensor.matmul(
                        po_[0 : DH + 1, ds(a, hi - a)],
                        v_sb[kt][:, h, :],
                        pt_g[kt][:, ds(a - qoff, hi - a)],
                        start=(kt == 0),
                        stop=(kt == stop_kt),
                    )

            def pv_norm(qc, h, pt_g, po_=None, skip_chain=False):
                ci, po = divmod(h, 2)
                po *= 64
                n_kt, width, off = tile_layout(qc)
                if po_ is None:
                    po_ = o_psum.tile([128, QC], F32, name="ov")
                if not skip_chain:
                    pv_chain(qc, h, pt_g, po_, range(n_kt))
                den = norm_pool.tile([64, QC], F32, name="den")
                row = norm_pool.tile([1, QC], F32, name="row")
                nc.vector.tensor_copy(row[:], po_[DH : DH + 1, :])
                nc.vector.reciprocal_approx_fast(den[0:1, :], row[:])
                nc.gpsimd.partition_broadcast(den[:], den[0:1, :])
                nc.vector.tensor_mul(
                    attnT_sb[ci][po : po + 64, ts(qc, QC)],
                    po_[0:DH, :],
                    den[:],
                )

            def out_proj_tile(qc, j, pool=None, tail=False):
                qt = qc * JPQ + j
                o_sb = out_pool.tile([128, D], BF16, name="osb")
                for ncol in range(2):
                    if pool is None:
                        pu = o_psum.tile([128, QC], F32, name="ov")
                    else:
                        # final chunk: the scores pool is idle by now; use
                        # its 3 wide slots so all 4 tail tiles overlap
                        pu = pool.tile([128, 2 * QC], F32, name="sc")[:, 0:QC]
                    for ci in range(2):
                        nc.tensor.matmul(
                            pu[:],
                            attnT_sb[ci][:, ts(qt, 128)],
                            wo_sb[:, ci, ts(ncol, 512)],
                            start=(ci == 0),
                            stop=(ci == 1),
                        )
                    # tail tiles: second half cast via scalar-engine Copy
                    # activation (same act table as Exp; scalar is idle in
                    # the tail) so the two casts run in parallel and the
                    # DMA issues sooner. gpsimd can't read PSUM.
                    if tail and ncol == 1:
                        nc.scalar.activation(
                            o_sb[:, ts(ncol, 512)],
                            pu[:],
                            mybir.ActivationFunctionType.Copy,
                        )
                    else:
                        nc.vector.tensor_copy(o_sb[:, ts(ncol, 512)], pu[:])
                ring = nc.sync if qt % 2 == 0 else nc.gpsimd
                ring.dma_start(out_ext.ap()[ts(qt, 128), :], o_sb[:])

            # ---------------- emission schedule ----------------
            # Engines execute strictly in program order, so PE bubbles in the
            # exp-latency-bound attention pipeline must be filled by
            # interleaving independent matmul work (projection chains for
            # q-chunk qc+1 and the previous chunk's out-projection) at
            # emission granularity.  The slot layout matches the original
            # per-head schedule; for qc 0-2 each per-head scores slot holds
            # half of that ci's PAIRED groups instead (both heads at once),
            # so exp/mask pacing per slot is unchanged while the PE time
            # halves.  qc3 stays unpaired: it is scalar(exp)-bound, so
            # pairing would only move the wait.
            c0 = proj_chunks(0)  # [Q0, K0, Q1, K1, V0..V3]
            pt_g0 = [[] for _ in range(HPC)]
            c0[0]()
            c0[1]()
            for kt in (0, 1):
                scores_pair_kt(0, 0, kt, pt_g0[0], pt_g0[1])
            c0[2]()
            c0[3]()
            for kt in (2, 3):
                scores_pair_kt(0, 0, kt, pt_g0[0], pt_g0[1])
            for emit in c0[4:]:
                emit()
            for qc in range(NQC):
                filler = proj_chunks(qc + 1) if qc + 1 < NQC else []
                fi = 0

                def fill(n):
                    nonlocal fi
                    for _ in range(n):
                        if fi < len(filler):
                            filler[fi]()
                            fi += 1

                def oprev(j):
                    if qc > 0:
                        out_proj_tile(qc - 1, j)

                pt_gs = pt_g0 if qc == 0 else [[] for _ in range(HPC)]
                ng = (qc + 1) * JPQ // 2
                paired = qc < NQC - 1
                if qc > 0:
                    # ci0 is paired for every chunk: per-kt pairing leaves
                    # the scalar exp data/call count unchanged, so even the
                    # exp-heavy final chunk nets the halved PE score time.
                    n_kt_qc = (qc + 1) * JPQ
                    for kt in range(n_kt_qc // 2):
                        scores_pair_kt(qc, 0, kt, pt_gs[0], pt_gs[1])
                    fill(2)
                    oprev(0)
                    for kt in range(n_kt_qc // 2, n_kt_qc):
                        scores_pair_kt(qc, 0, kt, pt_gs[0], pt_gs[1])
                    fill(1)
                else:
                    fill(3)
                pv_norm(qc, 0, pt_gs[0])
                fill(2)
                oprev(1)
                if paired:
                    n_kt_qc = (qc + 1) * JPQ
                    for kt in range(n_kt_qc // 2):
                        scores_pair_kt(qc, 1, kt, pt_gs[2], pt_gs[3])
                else:
                    scores(qc, 2, pt_gs[2])
                fill(1)
                pv_norm(qc, 1, pt_gs[1])
                fill(2)
                oprev(2)
                if qc == NQC - 1:
                    # tail: head 3's P@V runs as two column-slice chains (A =
                    # chunk cols [0,256), B = [256,512)) both pipelined into
                    # the score-group loop, so every PSUM write lands shortly
                    # after the last exp; head 2's full chain fills the
                    # remaining exp-latency bubbles. oprev(3) is emitted
                    # BEFORE po2/po3 so its o_psum slots pair with
                    # earlier (fully-read) tiles (slot-reuse WAR order).
                    oprev(3)
                    n_kt = (qc + 1) * JPQ
                    po2 = o_psum.tile([128, QC], F32, name="ov")
                    po3 = o_psum.tile([128, QC], F32, name="ov")
                    for g in range(n_kt // 2):
                        scores_group(qc, 3, pt_gs[3], g)
                        pv_chain(qc, 2, pt_gs[2], po2, [2 * g, 2 * g + 1])
                        if g >= 3:
                            pv_chain_slice(
                                qc, 3, pt_gs[3], po3,
                                [2 * g - 6, 2 * g - 5], 0, 256, n_kt - 3,
                            )
                    pv_chain_slice(
                        qc, 3, pt_gs[3], po3, range(10, n_kt - 2), 0, 256,
                        n_kt - 3,
                    )
                    # slice B needs its own PSUM bank: 'start' resets the
                    # whole bank's accumulation, so a second chain cannot
                    # share po3 with the (already accumulating) slice A.
                    # An s_psum tile allocated here pairs, in slot rotation,
                    # with a long-retired scores group.
                    ps_b = s_psum.tile([128, 2 * QC], F32, name="sc")[:, 0:QC]
                    pv_chain_slice(
                        qc, 3, pt_gs[3], ps_b, range(n_kt), 256, QC,
                        n_kt - 1,
                    )
                    # heads 2+3 normalize, emitted copies/recips first and
                    # muls last so the serial gpsimd broadcast chain overlaps
                    # the vector work and the PE's out-proj stream; head 3 is
                    # split in halves so its first mul lands sooner
                    den2 = norm_pool.tile([64, QC], F32, name="den")
                    row2 = norm_pool.tile([1, QC], F32, name="row")
                    den3 = norm_pool.tile([64, QC], F32, name="den")
                    row3 = norm_pool.tile([1, QC], F32, name="row")
                    nc.vector.tensor_copy(row2[:], po2[DH : DH + 1, :])
                    nc.vector.reciprocal_approx_fast(den2[0:1, :], row2[:])
                    nc.vector.tensor_copy(row3[:, 0:256], po3[DH : DH + 1, 0:256])
                    nc.vector.tensor_copy(
                        row3[:, 256:QC], ps_b[DH : DH + 1, 256:QC]
                    )
                    nc.vector.reciprocal_approx_fast(
                        den3[0:1, 0:256], row3[:, 0:256]
                    )
                    nc.vector.reciprocal_approx_fast(
                        den3[0:1, 256:QC], row3[:, 256:QC]
                    )
                    nc.gpsimd.partition_broadcast(den2[:], den2[0:1, :])
                    nc.gpsimd.partition_broadcast(
                        den3[:, 0:256], den3[0:1, 0:256]
                    )
                    nc.gpsimd.partition_broadcast(
                        den3[:, 256:QC], den3[0:1, 256:QC]
                    )
                    nc.vector.tensor_mul(
                        attnT_sb[1][0:64, ts(qc, QC)],
                        po2[0:DH, :],
                        den2[:],
                    )
                    nc.vector.tensor_mul(
                        attnT_sb[1][64:128, ds(qc * QC, 256)],
                        po3[0:DH, 0:256],
                        den3[:, 0:256],
                    )
                    nc.vector.tensor_mul(
                        attnT_sb[1][64:128, ds(qc * QC + 256, 256)],
                        ps_b[0:DH, 256:QC],
                        den3[:, 256:QC],
                    )
                    # tail out-proj in two phases: the ci0 half-contraction
                    # (heads 0/1, normalized long ago) streams right after
                    # chainB so the PE never idles (an idle gap here drops
                    # the PE to a half-rate p-state for the whole drain);
                    # the ci1 half lands as the head-2/3 normalizes complete.
                    # tiles 0-2: one wide s_psum tile each (2 banks = both
                    # ncol halves). tile 3: the two o_psum slots — their
                    # pending readers (the norm ops above) are already
                    # emitted, so slot-reuse WARs can't cycle with the PE.
                    pws = [
                        s_psum.tile([128, 2 * QC], F32, name="sc")
                        for _ in range(3)
                    ]
                    po_t3 = [
                        o_psum.tile([128, QC], F32, name="ov") for _ in range(2)
                    ]

                    def tail_pu(j, ncol):
                        if j < 3:
                            return pws[j][:, ts(ncol, QC)]
                        return po_t3[ncol][:]

                    o_sbs = [
                        out_pool.tile([128, D], BF16, name="osb")
                        for _ in range(JPQ)
                    ]
                    for j in range(JPQ):
                        qt = qc * JPQ + j
                        for ncol in range(2):
                            nc.tensor.matmul(
                                tail_pu(j, ncol),
                                attnT_sb[0][:, ts(qt, 128)],
                                wo_sb[:, 0, ts(ncol, 512)],
                                start=True,
                                stop=False,
                            )
                    for j in range(JPQ):
                        qt = qc * JPQ + j
                        for ncol in range(2):
                            pu = tail_pu(j, ncol)
                            nc.tensor.matmul(
                                pu,
                                attnT_sb[1][:, ts(qt, 128)],
                                wo_sb[:, 1, ts(ncol, 512)],
                                start=False,
                                stop=True,
                            )
                            if ncol == 1:
                                nc.scalar.activation(
                                    o_sbs[j][:, ts(ncol, 512)],
                                    pu,
                                    mybir.ActivationFunctionType.Copy,
                                )
                            else:
                                nc.vector.tensor_copy(
                                    o_sbs[j][:, ts(ncol, 512)], pu
                                )
                        ring = nc.sync if qt % 2 == 0 else nc.gpsimd
                        ring.dma_start(
                            out_ext.ap()[ts(qt, 128), :], o_sbs[j][:]
                        )
                else:
                    if paired:
                        n_kt_qc = (qc + 1) * JPQ
                        for kt in range(n_kt_qc // 2, n_kt_qc):
                            scores_pair_kt(qc, 1, kt, pt_gs[2], pt_gs[3])
                    else:
                        scores(qc, 3, pt_gs[3])
                    fill(1)
                    pv_norm(qc, 2, pt_gs[2])
                    fill(2)
                    oprev(3)
                    pv_norm(qc, 3, pt_gs[3])
                    fill(len(filler) - fi)

    nc.compile()
    return nc


_NC_CACHE = None


def _get_nc():
    global _NC_CACHE
    if _NC_CACHE is None:
        _NC_CACHE = build_nc()
    return _NC_CACHE


def make_in_maps(x, Wq, bq, Wk, bk, Wv, bv, Wo, bo):
    bf = ml_dtypes.bfloat16

    def qk_chunked(w):  # [1024, 256] -> [128, 2, 8, 128]
        return np.ascontiguousarray(
            w.reshape(NKC, 128, 2, 128).transpose(1, 2, 0, 3)
        ).astype(bf)

    in_maps = []
    for c in range(N_CORES):
        b, g = c // HPC, c % HPC
        lo, hi = g * DSL, (g + 1) * DSL
        xT = np.asarray(x[b]).T  # [1024, 2048]
        xf = xT[:, :QC].reshape(NKC, 128, QC).transpose(1, 0, 2)
        xr = (
            xT[:, QC:]
            .reshape(NKC, 128, NQC - 1, QC)
            .transpose(1, 2, 0, 3)  # [p, sc-1, kc, q]
        )
        bqk = np.stack(
            [bq[lo : lo + 128], bq[lo + 128 : hi], bk[lo : lo + 128],
             bk[lo + 128 : hi]],
            axis=1,
        )
        in_maps.append(
            {
                "xf": np.ascontiguousarray(xf).astype(bf),
                "xr": np.ascontiguousarray(xr).astype(bf),
                "wq": qk_chunked(Wq[:, lo:hi]),
                "wk": qk_chunked(Wk[:, lo:hi]),
                "wv": np.ascontiguousarray(
                    Wv[:, lo:hi].reshape(NKC, 128, DSL).transpose(1, 0, 2)
                ).astype(bf),
                "wo": np.ascontiguousarray(
                    Wo[lo:hi, :].reshape(2, 128, D).transpose(1, 0, 2)
                ).astype(bf),
                "bqk": np.ascontiguousarray(bqk).astype(np.float32),
            }
        )
    return in_maps


def gather_output(results, bv, Wo, bo):
    # softmax rows sum to 1, so the V-bias contributes bv @ Wo to every row
    corr = (np.asarray(bv, np.float64) @ np.asarray(Wo, np.float64)).astype(
        np.float32
    ) + np.asarray(bo, np.float32)
    out = np.empty((B, S, D), np.float32)
    for b in range(B):
        acc = np.zeros((S, D), np.float32)
        for g in range(HPC):
            acc += results[b * HPC + g]["out"].astype(np.float32)
        out[b] = acc + corr
    return out


def kernel(x, Wq, bq, Wk, bk, Wv, bv, Wo, bo, _trace=False):
    x = np.asarray(x, np.float32)
    nc = _get_nc()
    in_maps = make_in_maps(x, Wq, bq, Wk, bk, Wv, bv, Wo, bo)
    res = run_bass_kernel_spmd(nc, in_maps, list(range(N_CORES)), trace=_trace)
    out = gather_output(res.results, bv, Wo, bo)
    if _trace:
        return out, res
    return out
